# revision 1
# baseline (speedup 1.0000x reference)
"""MultiHeadAttention Trainium2 kernel (8 NeuronCores, SPMD).

Sharding: core c = (batch b=c//4, head-group g=c%4); each core owns 4 of 16
heads for one batch element. Wq/Wk/Wv are split by output features (tensor
parallel on heads), Wo by input features (row parallel); the 4 partial
[S, D] outputs per batch are summed on the host.

Schedule (v5): the attention kt-loop is software-pipelined with a lag-4
ctx stage (scores(kt+4)+exp issue before ctx(kt)) so the PE never waits
in-line on ACT's exp of the same key tile, and ACT — the attention-phase
bottleneck — streams exp back-to-back.  The softmax epilogue avoids the
DRAM bounce: denominator reciprocals (bf16) broadcast across each head's
64 partitions via a K=1 all-ones matmul into a borrowed score-PSUM bank,
ACT copies them to SBUF, DVE normalizes; odd heads shift to partitions
64:128 through an identity matmul so ct assembles as [128 feats, FT, QC]
for a K=128 output projection.  Q projections for q-chunks 1..3 and the
per-chunk output projections run out of a pending-work queue that drips
them into attention-phase PE slack a step at a time.  Projection bias
adds and PSUM->SBUF copies sit on DVE, exp and the bc copies on ACT.
"""

import numpy as np

B, S, D = 2, 2048, 1024
H, DK = 16, 64
HG = 4                 # heads per core
FC = HG * DK           # 256 features per core
NCORES = 8
P = 128
KSUB = D // P          # 8 contraction subtiles for projections
FT = FC // P           # 2 feature tiles (= head pairs)
NKT = S // P           # 16 key-position tiles
QC = 512               # q-chunk size
NQC = S // QC          # 4
SCHUNK = 512           # s-chunk for streaming projections
NSC = S // SCHUNK      # 4
LAG = 4                # ctx lags scores by LAG kt-steps

_PROGRAM = None        # cached Bass program - build once per process


def _build_program():
    from contextlib import ExitStack

    import concourse.bass as bass
    import concourse.mybir as mybir
    import concourse.tile as tile
    from concourse import bacc

    f32 = mybir.dt.float32
    bf16 = mybir.dt.bfloat16
    EXP = mybir.ActivationFunctionType.Exp
    IDENT = mybir.ActivationFunctionType.Identity

    nc = bacc.Bacc("TRN2", target_bir_lowering=False, debug=False)

    # consolidated inputs (2 per core): one bf16 blob + f32 biases.
    # blob[t, :, 0:S] = q/k/v activations (t=0,1,2); blob[t, :, S:S+FC]
    # their weight blocks; blob[3] carries Wo^T halves at cols
    # {0:1024, 1152:2176} of rows 0:128 and the 64x64 identity at
    # rows 0:64, cols 2176:2240.
    # blob[3, 128+p, 0:8] = per-partition f32 bias bytes (bq|bk) as bf16
    SW = S + FC            # 2304 blob columns
    qkvw = nc.dram_tensor("qkvw", [4, D, SW], bf16, kind="ExternalInput")
    out = nc.dram_tensor("out", [S, D], f32, kind="ExternalOutput")

    with tile.TileContext(nc) as tc, ExitStack() as ctx, nc.allow_low_precision(
        reason="bf16 matmul operands are intentional"
    ):
        weights = ctx.enter_context(tc.tile_pool(name="weights", bufs=1))
        instream = ctx.enter_context(tc.tile_pool(name="instream", bufs=5))
        persist = ctx.enter_context(tc.tile_pool(name="persist", bufs=1))
        exps = ctx.enter_context(tc.tile_pool(name="exps", bufs=10))
        ctpool = ctx.enter_context(tc.tile_pool(name="ctpool", bufs=3))
        bcast = ctx.enter_context(tc.tile_pool(name="bcast", bufs=2))
        small = ctx.enter_context(tc.tile_pool(name="small", bufs=2))
        outsb = ctx.enter_context(tc.tile_pool(name="outsb", bufs=3))
        ps_sc = ctx.enter_context(tc.tile_pool(name="ps_sc", bufs=2, space="PSUM"))
        ps_acc = ctx.enter_context(tc.tile_pool(name="ps_acc", bufs=4, space="PSUM"))

        # ---- persistent weights (wk first: K-proj of chunk 0 starts ASAP;
        # everything else queues behind the first kc chunk) ----
        qkvwr = qkvw[:, :, :].rearrange("t (o p) s -> p t o s", p=P)
        wor = qkvw[:, :, :].rearrange("t (o p) (f j) -> p t o f j", p=P, f=2)
        wk_sb = weights.tile([P, KSUB, FC], bf16, tag="wk")
        nc.sync.dma_start(wk_sb, qkvwr[:, 1, :, S:SW])
        wv_sb = weights.tile([P, KSUB, FC], bf16, tag="wv")
        wq_sb = weights.tile([P, KSUB, FC], bf16, tag="wq")
        wo_sb = weights.tile([P, FT, D], bf16, tag="wo")
        ones_sb = weights.tile([P, 64], bf16, tag="ones")
        nc.gpsimd.memset(ones_sb, 1.0)
        ident_sb = weights.tile([64, 64], bf16, tag="ident")
        braw = weights.tile([P, 8], bf16, tag="braw")
        bias_f = braw[:, :].bitcast(f32)   # [P, 4] = bq(2) | bk(2)

        # ---- persistent activations ----
        QT = persist.tile([P, FT, S], bf16, tag="QT")   # [feat, seq]
        KT = persist.tile([P, FT, S], bf16, tag="KT")   # [feat, seq]
        # V: [key, kt, h, 66] = [dims(0:64) | ones(64) | pad]; the ones
        # column accumulates the softmax denominator on ctx row 64.
        V = persist.tile([P, NKT, HG, 66], bf16, tag="V")
        nc.gpsimd.memset(V[:, :, :, 64:65], 1.0)

        # ---- K/V projections, streamed over s-chunks; Q projections last ----
        qTr = qkvwr[:, 0, :, 0:S]
        kTr = qkvwr[:, 1, :, 0:S]
        vTr = qkvwr[:, 2, :, 0:S]
        for c in range(NSC):
            sl = slice(c * SCHUNK, (c + 1) * SCHUNK)
            kc = instream.tile([P, KSUB, SCHUNK], bf16, tag="instream")
            nc.sync.dma_start(kc, kTr[:, :, sl])
            if c == 0:
                # deferred small/secondary weight loads queue behind kc0
                nc.sync.dma_start(braw, qkvwr[:, 3, 1, 0:8])
                nc.sync.dma_start(wv_sb, qkvwr[:, 2, :, S:SW])
            for ft in range(FT):
                ps = ps_acc.tile([P, SCHUNK], f32, tag="acc", name="kps")
                for ks in range(KSUB):
                    nc.tensor.matmul(
                        ps,
                        lhsT=wk_sb[:, ks, ft * P:(ft + 1) * P],
                        rhs=kc[:, ks, :],
                        start=(ks == 0),
                        stop=(ks == KSUB - 1),
                    )
                nc.vector.tensor_scalar_add(KT[:, ft, sl], ps, bias_f[:, 2 + ft:3 + ft])
            vc = instream.tile([P, KSUB, SCHUNK], bf16, tag="instream")
            nc.sync.dma_start(vc, vTr[:, :, sl])
            for st in range(SCHUNK // P):
                ps = ps_acc.tile([P, SCHUNK], f32, tag="acc", name="vps")
                for ks in range(KSUB):
                    nc.tensor.matmul(
                        ps[:, :FC],
                        lhsT=vc[:, ks, st * P:(st + 1) * P],
                        rhs=wv_sb[:, ks, :],
                        start=(ks == 0),
                        stop=(ks == KSUB - 1),
                    )
                kt_idx = c * (SCHUNK // P) + st
                # ACT is idle during the projection phase; keep DVE clear
                # for the K/Q bias-adds the first scores wait on
                nc.scalar.activation(
                    V[:, kt_idx, :, 0:DK],
                    ps[:, 0:FC].rearrange("p (h d) -> p h d", h=HG),
                    IDENT,
                )
        nc.sync.dma_start(wq_sb, qkvwr[:, 0, :, S:SW])
        nc.sync.dma_start(wo_sb, wor[:, 3, 0, :, 0:D])
        nc.sync.dma_start(ident_sb, qkvwr[0:64, 3, 0, 2176:2240])

        # Q projection: chunk 0 up front (attention qc=0 needs it); chunks
        # 1..3 interleave into the attention phase's PE slack (emit_qproj
        # via the pending queue).
        def emit_qproj_dma(c):
            sl = slice(c * SCHUNK, (c + 1) * SCHUNK)
            qc_t = instream.tile([P, KSUB, SCHUNK], bf16, tag="instream",
                                 name="qc_t")
            nc.sync.dma_start(qc_t, qTr[:, :, sl])
            return qc_t

        def emit_qproj_compute(c, qc_t, qps_views):
            # qps_views: per-ft single-bank fp32 PSUM views [P, SCHUNK]
            sl = slice(c * SCHUNK, (c + 1) * SCHUNK)
            for ft in range(FT):
                for ks in range(KSUB):
                    nc.tensor.matmul(
                        qps_views[ft],
                        lhsT=wq_sb[:, ks, ft * P:(ft + 1) * P],
                        rhs=qc_t[:, ks, :],
                        start=(ks == 0),
                        stop=(ks == KSUB - 1),
                    )
            for ft in range(FT):
                nc.vector.tensor_scalar_add(
                    QT[:, ft, sl], qps_views[ft], bias_f[:, ft:ft + 1]
                )

        qt0 = emit_qproj_dma(0)
        qps0 = [
            ps_acc.tile([P, SCHUNK], f32, tag="acc", name="qps0"),
            ps_acc.tile([P, SCHUNK], f32, tag="acc", name="qps1"),
        ]
        emit_qproj_compute(0, qt0, qps0)

        # ---- attention + output projection: lag-LAG software pipeline ----
        # per (qc, kt) step: emit scores+exp; ctx trails LAG steps behind;
        # the epilogue for qc emits right after ctx(qc, NKT-1).
        ctxu = {}        # qc -> [4 PSUM accumulators]
        evq = {}         # (qc, kt) -> [e_ft0, e_ft1]

        def emit_scores(qc, kt):
            qsl = slice(qc * QC, (qc + 1) * QC)
            ksl = slice(kt * P, (kt + 1) * P)
            evq[(qc, kt)] = []
            for ft in range(FT):
                sc = ps_sc.tile([P, 2 * QC], f32, tag="sc", name="sc")
                nc.tensor.matmul(
                    sc[:, 0:QC],
                    lhsT=KT[0:64, ft, ksl],
                    rhs=QT[0:64, ft, qsl],
                    start=True, stop=True,
                    tile_position=(0, 0),
                )
                nc.tensor.matmul(
                    sc[:, QC:2 * QC],
                    lhsT=KT[64:128, ft, ksl],
                    rhs=QT[64:128, ft, qsl],
                    start=True, stop=True,
                    tile_position=(64, 0),
                )
                e = exps.tile([P, 2 * QC], bf16, tag="exps", name="e")
                nc.scalar.activation(e, sc, EXP)
                evq[(qc, kt)].append(e)

        def emit_ctx(qc, kt):
            first, last = kt == 0, kt == NKT - 1
            if first:
                ctxu[qc] = [
                    ps_acc.tile([P, QC], f32, tag="acc", name=f"ctxu{hh}")
                    for hh in range(HG)
                ]
            ex = evq.pop((qc, kt))
            for h in range(HG):
                # rows 0:64 = unnormalized ctx^T dims, row 64 = denominator
                nc.tensor.matmul(
                    ctxu[qc][h][0:65, :],
                    lhsT=V[:, kt, h, 0:65],
                    rhs=ex[h // 2][:, (h % 2) * QC:(h % 2 + 1) * QC],
                    start=first, stop=last,
                )

        cts = {}         # qc -> normalized per-head ct tile

        def emit_epilogue_a(qc):
            # Normalize: reciprocal of row-64 denominators (DVE, bf16),
            # broadcast each across 64 partitions via a K=1 ones-matmul
            # (PE -> borrowed score-PSUM bank), copy to SBUF, multiply.
            # Odd heads' normalized ct shifts to partitions 64:128 via an
            # identity matmul so ct assembles as [128 feats, FT, QC].
            cu = ctxu.pop(qc)
            recip = small.tile([P, HG, QC], bf16, tag="recip", name="recip")
            bc = bcast.tile([64, HG, QC], f32, tag="bcast", name="bc")
            ct = ctpool.tile([P, FT, QC], bf16, tag="ct", name="ct")
            for h in range(HG):
                nc.vector.reciprocal(recip[64:65, h, :], cu[h][64:65, :])
            bct = [
                ps_sc.tile([P, 2 * QC], f32, tag="sc", name="bct0"),
                ps_sc.tile([P, 2 * QC], f32, tag="sc", name="bct1"),
            ]
            for h in range(HG):
                t, col = divmod(h, 2)
                nc.tensor.matmul(
                    bct[t][0:64, col * QC:(col + 1) * QC],
                    lhsT=ones_sb[64:65, :],
                    rhs=recip[64:65, h, :],
                    start=True, stop=True,
                )
            for h in range(HG):
                t, col = divmod(h, 2)
                nc.scalar.activation(
                    bc[:, h, :], bct[t][0:64, col * QC:(col + 1) * QC], IDENT
                )
            cttmp = small.tile([64, FT, QC], bf16, tag="cttmp", name="cttmp")
            for h in range(HG):
                ft, odd = divmod(h, 2)
                dst = ct[0:64, ft, :] if not odd else cttmp[:, ft, :]
                nc.vector.tensor_mul(out=dst, in0=cu[h][0:64, :], in1=bc[:, h, :])
            for ft in range(FT):
                # partition shift 0:64 -> 64:128 through the PE
                t = ft
                nc.tensor.matmul(
                    bct[t][64:128, 0:QC],
                    lhsT=ident_sb,
                    rhs=cttmp[:, ft, :],
                    start=True, stop=True,
                )
                nc.vector.tensor_copy(
                    out=ct[64:128, ft, :], in_=bct[t][64:128, 0:QC]
                )
            cts[qc] = ct

        def emit_epilogue_b(qc, st):
            # K=128 output projection for one 128-row q tile
            ct = cts[qc]
            s0 = qc * QC + st * P
            ops = ps_sc.tile([P, 2 * QC], f32, tag="sc", name="ops")
            for jc in range(D // 512):
                for ft in range(FT):
                    nc.tensor.matmul(
                        ops[:, jc * QC:(jc + 1) * QC],
                        lhsT=ct[:, ft, st * P:(st + 1) * P],
                        rhs=wo_sb[:, ft, jc * 512:(jc + 1) * 512],
                        start=(ft == 0),
                        stop=(ft == FT - 1),
                    )
            osb = outsb.tile([P, D], f32, tag="osb", name="osb")
            nc.vector.tensor_copy(out=osb[:, :], in_=ops[:, :])
            nc.sync.dma_start(out[s0:s0 + P, :], osb)
            if st == QC // P - 1:
                cts.pop(qc)

        def emit_qproj_pop(c, qc_t):
            # whole Q chunk in one PE block; both ft chains share one
            # borrowed 2-bank score-PSUM tile (independent banks)
            qps = ps_sc.tile([P, 2 * QC], f32, tag="sc", name="qps")
            emit_qproj_compute(
                c, qc_t, [qps[:, 0:SCHUNK], qps[:, SCHUNK:2 * SCHUNK]]
            )

        from collections import deque

        pending = deque()
        steps = [(qc, kt) for qc in range(NQC) for kt in range(NKT)]
        qt_next = None
        for i, (qc, kt) in enumerate(steps):
            if kt == 0 and qc + 1 < NQC:
                # prefetch next q-chunk's inputs; project into PE slack later
                qt_next = emit_qproj_dma(qc + 1)
            if kt == 3 and qt_next is not None:
                pending.append(
                    lambda c=qc + 1, t=qt_next: emit_qproj_pop(c, t)
                )
                qt_next = None
            emit_scores(qc, kt)
            if pending:
                pending.popleft()()
            if i >= LAG:
                pqc, pkt = steps[i - LAG]
                emit_ctx(pqc, pkt)
                if pkt == NKT - 1:
                    emit_epilogue_a(pqc)
                    # delay out-proj pops 2 steps so ct4 is ready when the
                    # PE reaches the first out-proj matmul, then spread the
                    # 4 st tiles across the chunk's steps
                    pending.append(lambda: None)
                    pending.append(lambda: None)
                    for st in range(QC // P):
                        pending.append(
                            lambda pqc=pqc, st=st: emit_epilogue_b(pqc, st)
                        )
        for j in range(len(steps) - LAG, len(steps)):
            pqc, pkt = steps[j]
            emit_ctx(pqc, pkt)
            if pending:
                pending.popleft()()
            if pkt == NKT - 1:
                emit_epilogue_a(pqc)
                for st in range(QC // P):
                    pending.append(
                        lambda pqc=pqc, st=st: emit_epilogue_b(pqc, st)
                    )
        while pending:
            pending.popleft()()

    nc.compile()
    return nc


def _get_program():
    global _PROGRAM
    if _PROGRAM is None:
        _PROGRAM = _build_program()
    return _PROGRAM


def _host_shards(q, k, v, Wq, bq, Wk, bk, Wv, bv, Wo, bo):
    """Build the 8 per-core input dicts (host-side transposes/slices)."""
    import ml_dtypes

    b16 = ml_dtypes.bfloat16
    scale = 1.0 / np.sqrt(np.float32(DK))
    qT = [np.ascontiguousarray(q[b].T).astype(b16) for b in range(B)]
    kT = [np.ascontiguousarray(k[b].T).astype(b16) for b in range(B)]
    vT = [np.ascontiguousarray(v[b].T).astype(b16) for b in range(B)]
    in_maps = []
    for c in range(NCORES):
        b, g = divmod(c, NCORES // B)
        fsl = slice(g * FC, (g + 1) * FC)
        blob = np.zeros((4, D, S + FC), dtype=b16)
        blob[0, :, 0:S] = qT[b]
        blob[1, :, 0:S] = kT[b]
        blob[2, :, 0:S] = vT[b]
        blob[0, :, S:] = (Wq[fsl, :].T * scale).astype(b16)
        blob[1, :, S:] = Wk[fsl, :].T.astype(b16)
        blob[2, :, S:] = Wv[fsl, :].T.astype(b16)
        woTb = Wo[:, fsl].T.astype(b16)          # [FC, D]
        blob[3, 0:P, 0:D] = woTb[0:P, :]
        blob[3, 0:P, 1152:1152 + D] = woTb[P:2 * P, :]
        blob[3, 0:64, 2176:2240] = np.eye(64, dtype=b16)
        bqs = (bq[fsl] * scale).astype(np.float32)
        bks = bk[fsl].astype(np.float32)
        bpack = np.stack([bqs[0:P], bqs[P:2 * P], bks[0:P], bks[P:2 * P]],
                         axis=1)                       # [P, 4] f32
        blob[3, P:2 * P, 0:8] = np.ascontiguousarray(bpack).view(b16)
        in_maps.append({"qkvw": blob})
    return in_maps


def kernel(q, k, v, mask, Wq, bq, Wk, bk, Wv, bv, Wo, bo):
    q = np.asarray(q, dtype=np.float32)
    k = np.asarray(k, dtype=np.float32)
    v = np.asarray(v, dtype=np.float32)
    mask = np.asarray(mask)
    Wq = np.asarray(Wq, dtype=np.float32)
    bq = np.asarray(bq, dtype=np.float32)
    Wk = np.asarray(Wk, dtype=np.float32)
    bk = np.asarray(bk, dtype=np.float32)
    Wv = np.asarray(Wv, dtype=np.float32)
    bv = np.asarray(bv, dtype=np.float32)
    Wo = np.asarray(Wo, dtype=np.float32)
    bo = np.asarray(bo, dtype=np.float32)

    if not np.all(mask != 0):
        # Unmasked-path kernel; fall back to exact host computation if a
        # nontrivial mask ever shows up (spec fills the mask with ones).
        return _host_reference(q, k, v, mask, Wq, bq, Wk, bk, Wv, bv, Wo, bo)

    from concourse.bass_utils import run_bass_kernel_spmd

    nc = _get_program()
    in_maps = _host_shards(q, k, v, Wq, bq, Wk, bk, Wv, bv, Wo, bo)
    res = run_bass_kernel_spmd(nc, in_maps, core_ids=list(range(NCORES)))

    # host reduction: sum the 4 row-parallel Wo partials per batch,
    # then add the exact bv/bo correction (softmax rows sum to 1).
    const = bv @ Wo.T + bo
    out = np.empty((B, S, D), np.float32)
    gpb = NCORES // B
    for b in range(B):
        acc = res.results[b * gpb]["out"].astype(np.float32)
        for g in range(1, gpb):
            acc = acc + res.results[b * gpb + g]["out"]
        out[b] = acc + const[None, :]
    return out


def _host_reference(q, k, v, mask, Wq, bq, Wk, bk, Wv, bv, Wo, bo):
    def split_heads(x):
        b, s, _ = x.shape
        return x.reshape(b, s, H, DK).transpose(0, 2, 1, 3)

    query = split_heads(q @ Wq.T + bq)
    key_ = split_heads(k @ Wk.T + bk)
    value = split_heads(v @ Wv.T + bv)
    scores = np.einsum("bhqd,bhkd->bhqk", query, key_) / np.sqrt(np.float32(DK))
    scores = np.where(mask == 0, np.float32(-1e9), scores).astype(np.float32)
    scores -= scores.max(axis=-1, keepdims=True)
    e = np.exp(scores)
    attn = e / e.sum(axis=-1, keepdims=True)
    ctx = np.einsum("bhqk,bhkd->bhqd", attn, value)
    ctx = ctx.transpose(0, 2, 1, 3).reshape(q.shape[0], -1, D)
    return (ctx @ Wo.T + bo).astype(np.float32)

